# revision 9
# baseline (speedup 1.0000x reference)
"""Bag self-attention kernel for TRN2, data-parallel over the bag dim (8 cores).

Per core (one bag, x: [N=2048, L=1280], H=160):
  q = x@Wq.T + bq ; k = x@Wk.T (bk cancels in softmax) ; v = x@Wv.T
  S = q@k.T ; P = softmax(S) ; out = P@v + (x + bv)      (gamma = 1)

Device-side layout choices:
  - All matmuls run in float32r (TF32-grade, full PE rate at free-dim >= 256).
  - Host pre-transposes x->xT, Wq->WqT, Wk->WkT, Wv->WvT so no on-device
    transposes are needed (sharding is host-side anyway).
  - Attention is computed in the ST=[j,i] layout: ST = kT.T-chunks x qT,
    E = exp(ST) directly feeds P@v as the stationary operand, so the usual
    P-transpose disappears. Softmax runs without max-subtraction (energies
    are ~N(0,160); exp is exact to ~1e-5 over [-90, 80] on ACT) and the
    normalizer Z_i comes for free as a ones-column appended to v.
  - v-bias bv rides the residual (sum_j P == 1), k-bias drops entirely.
"""

import numpy as np

import concourse.bass as bass
import concourse.mybir as mybir
import concourse.tile as tile
from concourse import bacc
from concourse.bass_utils import run_bass_kernel_spmd

B, N, L, H = 8, 2048, 1280, 160
f32 = mybir.dt.float32
f32r = mybir.dt.float32r

NL = L // 128            # 10 l-chunks (contraction for projections)
NJ = N // 128            # 16 j-chunks
NI = N // 512            # 4 i-chunks
MQ = 4                   # WvT window quarters (m = 320 each)
MQW = L // MQ            # 320
H0, H1 = 128, H - 128    # h-chunks 128 + 32
O_CHUNKS = [(0, 512), (512, 1024), (1024, L + 2)]  # last chunk carries 2 Z cols (f32r needs even free dim)


def _build():
    nc = bacc.Bacc()
    xT_d = nc.declare_dram_parameter("xT", [L, N], f32r, isOutput=False)
    xr_d = nc.declare_dram_parameter("xresid", [N, L], f32, isOutput=False)
    wq_d = nc.declare_dram_parameter("WqT", [L, H0], f32r, isOutput=False)
    wk_d = nc.declare_dram_parameter("WkT", [L, H0], f32r, isOutput=False)
    wqk1_d = nc.declare_dram_parameter("Wqk1T", [L, 2 * H1], f32r, isOutput=False)
    wv_d = nc.declare_dram_parameter("WvT", [L, L], f32r, isOutput=False)
    bq_d = nc.declare_dram_parameter("bq", [H], f32, isOutput=False)
    ones_d = nc.declare_dram_parameter("ones", [128, 2], f32r, isOutput=False)
    out_d = nc.declare_dram_parameter("out", [N, L], f32, isOutput=True)

    with tile.TileContext(nc) as tc:
        with (
            tc.tile_pool(name="const", bufs=1) as constp,
            tc.tile_pool(name="vprime", bufs=1) as vpool,
            tc.tile_pool(name="qkt", bufs=1) as qktp,
        ):
            bq_t = [constp.tile([H0, 1], f32, tag="bq0", name="bq0"),
                    constp.tile([H1, 1], f32, tag="bq1", name="bq1")]
            nc.sync.dma_start(out=bq_t[0], in_=bq_d[0:H0].unsqueeze(1))
            nc.sync.dma_start(out=bq_t[1], in_=bq_d[H0:H].unsqueeze(1))

            # v' tiles: [128, L+1] per j-chunk; col L is the ones column for Z
            v_t = [vpool.tile([128, L + 2], f32r, tag=f"v{j}", name=f"v{j}") for j in range(NJ)]
            for j in range(NJ):
                nc.sync.dma_start(out=v_t[j][:, L:L + 2], in_=ones_d[:])

            # qT/kT resident: [160, 2048] as 128+32 partition tiles each
            qT = [qktp.tile([H0, N], f32r, tag="q0", name="q0"),
                  qktp.tile([H1, N], f32r, tag="q1", name="q1")]
            kT = [qktp.tile([H0, N], f32r, tag="k0", name="k0"),
                  qktp.tile([H1, N], f32r, tag="k1", name="k1")]

            # Weights loaded once (per-l tiles so the first matmul gates on
            # ~320KB of DMA, not 6MB); xT consumed in 4 column-generations
            # (gen g: i-chunk g, j-chunks 4g..4g+3), tags double-buffered and
            # the next generation prefetched before WvT streaming starts.
            wv_r = wv_d.rearrange("(c p) m -> p c m", p=128)
            with tc.tile_pool(name="wqk", bufs=1) as wqkp:

                def dma_x_tile(g, l):
                    c0 = g * 512
                    t = xtp.tile([128, 512], f32r, tag=f"x{l}", name=f"x{g}_{l}")
                    nc.sync.dma_start(
                        out=t, in_=xT_d[l * 128:(l + 1) * 128, c0:c0 + 512])
                    return t

                with tc.tile_pool(name="xt", bufs=2, side="right") as xtp:
                    # interleave gen-0 x tiles with the weight tiles so the
                    # first accumulation group's operands land first
                    wq_t, wk_t, wqk1_t, x_t = [], [], [], []
                    for l in range(NL):
                        x_t.append(dma_x_tile(0, l))
                        wq = wqkp.tile([128, H0], f32r, tag=f"wq{l}", name=f"wq{l}")
                        wk = wqkp.tile([128, H0], f32r, tag=f"wk{l}", name=f"wk{l}")
                        w1 = wqkp.tile([128, 2 * H1], f32r, tag=f"wqk1{l}", name=f"wqk1{l}")
                        nc.sync.dma_start(out=wq, in_=wq_d[l * 128:(l + 1) * 128, :])
                        nc.sync.dma_start(out=wk, in_=wk_d[l * 128:(l + 1) * 128, :])
                        nc.sync.dma_start(out=w1, in_=wqk1_d[l * 128:(l + 1) * 128, :])
                        wq_t.append(wq); wk_t.append(wk); wqk1_t.append(w1)

                    for g in range(4):
                        c0 = g * 512
                        if g > 0:
                            x_t = x_next  # noqa: F821

                        # q/k projections for i-chunk g
                        isl = slice(c0, c0 + 512)
                        with tc.tile_pool(name="qkps", bufs=1, space="PSUM") as qkps:
                            ps = qkps.tile([H0, 512], f32, tag="qps", name=f"qps{g}")
                            for l in range(NL):
                                nc.tensor.matmul(ps, wq_t[l], x_t[l],
                                                 start=(l == 0), stop=(l == NL - 1))
                            nc.vector.tensor_scalar_add(qT[0][:, isl], ps, bq_t[0])
                            ps2 = qkps.tile([H0, 512], f32, tag="kps", name=f"kps{g}")
                            for l in range(NL):
                                nc.tensor.matmul(ps2, wk_t[l], x_t[l],
                                                 start=(l == 0), stop=(l == NL - 1))
                            nc.any.tensor_copy(kT[0][:, isl], ps2)
                            ps3 = qkps.tile([2 * H1, 512], f32, tag="qk1ps",
                                            name=f"qk1ps{g}")
                            for l in range(NL):
                                nc.tensor.matmul(ps3, wqk1_t[l], x_t[l],
                                                 start=(l == 0), stop=(l == NL - 1))
                            nc.vector.tensor_scalar_add(qT[1][:, isl],
                                                        ps3[0:H1, :], bq_t[1])
                            nc.any.tensor_copy(kT[1][:, isl], ps3[H1:2 * H1, :])

                        # prefetch next generation's xT before WvT streaming
                        if g < 3:
                            x_next = [dma_x_tile(g + 1, l) for l in range(NL)]

                        # v-projection for j-chunks 4g..4g+3, m in quarters
                        with (
                            tc.tile_pool(name="wv", bufs=3, side="right") as wvp,
                            tc.tile_pool(name="vps", bufs=4, space="PSUM") as vps,
                        ):
                            for mq in range(MQ):
                                wv_t = wvp.tile([128, NL, MQW], f32r, tag="wv",
                                                name=f"wv{g}_{mq}")
                                nc.sync.dma_start(
                                    out=wv_t,
                                    in_=wv_r[:, :, mq * MQW:(mq + 1) * MQW])
                                for j in range(4 * g, 4 * g + 4):
                                    ps = vps.tile([128, MQW], f32, tag="vps",
                                                  name=f"vps{g}_{mq}_{j}")
                                    jloc = j * 128 - c0
                                    for l in range(NL):
                                        nc.tensor.matmul(
                                            ps,
                                            x_t[l][:, jloc:jloc + 128],
                                            wv_t[:, l, :],
                                            start=(l == 0), stop=(l == NL - 1))
                                    nc.any.tensor_copy(
                                        v_t[j][:, mq * MQW:(mq + 1) * MQW], ps)

            # ---- Phase 3: attention, i-chunk at a time (xT pool is closed)
            with (
                tc.tile_pool(name="ep", bufs=1) as ep,
                tc.tile_pool(name="stg", bufs=3) as stg,
                tc.tile_pool(name="sps", bufs=2, space="PSUM") as sps,
                tc.tile_pool(name="ops", bufs=2, space="PSUM") as ops,
            ):
                for ic in range(NI):
                    isl = slice(ic * 512, (ic + 1) * 512)
                    e_t = []
                    for j in range(NJ):
                        jsl = slice(j * 128, (j + 1) * 128)
                        sp = sps.tile([128, 512], f32, tag="sps")
                        nc.tensor.matmul(sp, kT[0][:, jsl], qT[0][:, isl],
                                         start=True, stop=False)
                        nc.tensor.matmul(sp, kT[1][:, jsl], qT[1][:, isl],
                                         start=False, stop=True)
                        et = ep.tile([128, 512], f32r, tag=f"E{j}")
                        nc.scalar.activation(et, sp,
                                             mybir.ActivationFunctionType.Exp)
                        e_t.append(et)
                    for isub in range(4):
                        i0 = ic * 512 + isub * 128
                        op_t = ops.tile([128, L + 2], f32, tag="ops")
                        esl = slice(isub * 128, (isub + 1) * 128)
                        for j in range(NJ):
                            for (mlo, mhi) in O_CHUNKS:
                                nc.tensor.matmul(
                                    op_t[:, mlo:mhi], e_t[j][:, esl],
                                    v_t[j][:, mlo:mhi],
                                    start=(j == 0), stop=(j == NJ - 1))
                        recip = stg.tile([128, 1], f32, tag="recip")
                        nc.vector.reciprocal(recip, op_t[:, L:L + 1])
                        xr = stg.tile([128, L], f32, tag="xr")
                        nc.sync.dma_start(out=xr, in_=xr_d[i0:i0 + 128, :])
                        ot = stg.tile([128, L], f32, tag="out")
                        nc.vector.scalar_tensor_tensor(
                            out=ot, in0=op_t[:, 0:L], scalar=recip, in1=xr,
                            op0=mybir.AluOpType.mult, op1=mybir.AluOpType.add)
                        nc.sync.dma_start(out=out_d[i0:i0 + 128, :], in_=ot)

    nc.finalize()
    return nc


_NC = None


def _get_nc():
    global _NC
    if _NC is None:
        _NC = _build()
    return _NC


def kernel(x, Wq, bq, Wk, bk, Wv, bv):
    x = np.asarray(x, dtype=np.float32)
    WqT_full = np.asarray(Wq, np.float32).T                    # [L, H]
    WkT_full = np.asarray(Wk, np.float32).T                    # [L, H]
    WqT = np.ascontiguousarray(WqT_full[:, :H0])               # [L, 128]
    WkT = np.ascontiguousarray(WkT_full[:, :H0])               # [L, 128]
    Wqk1T = np.ascontiguousarray(
        np.concatenate([WqT_full[:, H0:], WkT_full[:, H0:]], axis=1))  # [L, 64]
    WvT = np.ascontiguousarray(np.asarray(Wv, np.float32).T)   # [L, L]
    bq = np.asarray(bq, np.float32)
    bv = np.asarray(bv, np.float32)

    nc = _get_nc()
    in_maps = []
    for b in range(B):
        in_maps.append({
            "xT": np.ascontiguousarray(x[b].T),
            "xresid": x[b] + bv[None, :],
            "WqT": WqT,
            "WkT": WkT,
            "Wqk1T": Wqk1T,
            "WvT": WvT,
            "bq": bq,
            "ones": np.ones((128, 2), np.float32),
        })
    res = run_bass_kernel_spmd(nc, in_maps, list(range(B)))
    return np.stack([res.results[b]["out"] for b in range(B)], axis=0)


if __name__ == "__main__":
    rng = np.random.default_rng(0)
    ins = {
        "x": rng.standard_normal((B, N, L)).astype(np.float32),
        "Wq": rng.standard_normal((H, L)).astype(np.float32) * 0.028,
        "bq": rng.standard_normal((H,)).astype(np.float32) * 0.028,
        "Wk": rng.standard_normal((H, L)).astype(np.float32) * 0.028,
        "bk": rng.standard_normal((H,)).astype(np.float32) * 0.028,
        "Wv": rng.standard_normal((L, L)).astype(np.float32) * 0.028,
        "bv": rng.standard_normal((L,)).astype(np.float32) * 0.028,
    }
    out = kernel(**ins)
    print("kernel ran, out shape", out.shape)


# revision 10
# speedup vs baseline: 1.2508x; 1.2508x over previous
"""Bag self-attention kernel for TRN2, data-parallel over the bag dim (8 cores).

Per core (one bag, x: [N=2048, L=1280], H=160):
  q = x@Wq.T + bq ; k = x@Wk.T (bk cancels in softmax) ; v = x@Wv.T
  S = q@k.T ; P = softmax(S) ; out = P@v + (x + bv)      (gamma = 1)

Device-side layout choices:
  - All matmuls run in float32r (TF32-grade, full PE rate at free-dim >= 256).
  - Host pre-transposes x->xT, Wq->WqT, Wk->WkT, Wv->WvT so no on-device
    transposes are needed (sharding is host-side anyway).
  - Attention is computed in the ST=[j,i] layout: ST = kT.T-chunks x qT,
    E = exp(ST) directly feeds P@v as the stationary operand, so the usual
    P-transpose disappears. Softmax runs without max-subtraction (energies
    are ~N(0,160); exp is exact to ~1e-5 over [-90, 80] on ACT) and the
    normalizer Z_i comes for free as a ones-column appended to v.
  - v-bias bv rides the residual (sum_j P == 1), k-bias drops entirely.
"""

import numpy as np

import concourse.bass as bass
import concourse.mybir as mybir
import concourse.tile as tile
from concourse import bacc
from concourse.bass_utils import run_bass_kernel_spmd

B, N, L, H = 8, 2048, 1280, 160
f32 = mybir.dt.float32
f32r = mybir.dt.float32r

NL = L // 128            # 10 l-chunks (contraction for projections)
NJ = N // 128            # 16 j-chunks
NI = N // 512            # 4 i-chunks
MQ = 4                   # WvT window quarters (m = 320 each)
MQW = L // MQ            # 320
H0, H1 = 128, H - 128    # h-chunks 128 + 32
O_CHUNKS = [(0, 512), (512, 1024), (1024, L + 2)]  # last chunk carries 2 Z cols (f32r needs even free dim)


def _build():
    nc = bacc.Bacc()
    xT_d = nc.declare_dram_parameter("xT", [L, N], f32r, isOutput=False)
    xr_d = nc.declare_dram_parameter("xresid", [N, L], f32, isOutput=False)
    wq_d = nc.declare_dram_parameter("WqT", [L, H0], f32r, isOutput=False)
    wk_d = nc.declare_dram_parameter("WkT", [L, H0], f32r, isOutput=False)
    wqk1_d = nc.declare_dram_parameter("Wqk1T", [L, 2 * H1], f32r, isOutput=False)
    wv_d = nc.declare_dram_parameter("WvT", [L, L], f32r, isOutput=False)
    bq_d = nc.declare_dram_parameter("bq", [H], f32, isOutput=False)
    ones_d = nc.declare_dram_parameter("ones", [128, 2], f32r, isOutput=False)
    out_d = nc.declare_dram_parameter("out", [N, L], f32, isOutput=True)

    with tile.TileContext(nc) as tc:
        with (
            tc.tile_pool(name="const", bufs=1) as constp,
            tc.tile_pool(name="vprime", bufs=1) as vpool,
            tc.tile_pool(name="qkt", bufs=1) as qktp,
        ):
            bq_t = [constp.tile([H0, 1], f32, tag="bq0", name="bq0"),
                    constp.tile([H1, 1], f32, tag="bq1", name="bq1")]
            nc.sync.dma_start(out=bq_t[0], in_=bq_d[0:H0].unsqueeze(1))
            nc.sync.dma_start(out=bq_t[1], in_=bq_d[H0:H].unsqueeze(1))

            # v' tiles: [128, L+1] per j-chunk; col L is the ones column for Z
            v_t = [vpool.tile([128, L + 2], f32r, tag=f"v{j}", name=f"v{j}") for j in range(NJ)]
            for j in range(NJ):
                nc.sync.dma_start(out=v_t[j][:, L:L + 2], in_=ones_d[:])

            # qT/kT resident: [160, 2048] as 128+32 partition tiles each
            qT = [qktp.tile([H0, N], f32r, tag="q0", name="q0"),
                  qktp.tile([H1, N], f32r, tag="q1", name="q1")]
            kT = [qktp.tile([H0, N], f32r, tag="k0", name="k0"),
                  qktp.tile([H1, N], f32r, tag="k1", name="k1")]

            # xT consumed in 4 column-generations (gen g: i-chunk g,
            # j-chunks 4g..4g+3); x tiles double-buffered and prefetched one
            # generation ahead. The model serializes DMA transfers, so DMAs
            # are emitted in critical-path order: gen0 weights+x first, then
            # WvT quarters pace the v-projection loop.
            wv_r = wv_d.rearrange("(c p) m -> p c m", p=128)
            with (
                tc.tile_pool(name="wqk", bufs=1) as wqkp,
                tc.tile_pool(name="xt", bufs=2, side="right") as xtp,
                tc.tile_pool(name="wv", bufs=3, side="right") as wvp,
                tc.tile_pool(name="vps", bufs=4, space="PSUM") as vps,
                tc.tile_pool(name="qkps", bufs=1, space="PSUM") as qkps,
            ):
                def dma_x_gen(g):
                    c0 = g * 512
                    ts = []
                    for l in range(NL):
                        t = xtp.tile([128, 512], f32r, tag=f"x{l}", name=f"x{g}_{l}")
                        nc.sync.dma_start(
                            out=t, in_=xT_d[l * 128:(l + 1) * 128, c0:c0 + 512])
                        ts.append(t)
                    return ts

                def qk_proj(g, x_t):
                    isl = slice(g * 512, (g + 1) * 512)
                    ps = qkps.tile([H0, 512], f32, tag="qps", name=f"qps{g}")
                    for l in range(NL):
                        nc.tensor.matmul(ps, wq_t[:, l, :], x_t[l],
                                         start=(l == 0), stop=(l == NL - 1))
                    nc.vector.tensor_scalar_add(qT[0][:, isl], ps, bq_t[0])
                    ps2 = qkps.tile([H0, 512], f32, tag="kps", name=f"kps{g}")
                    for l in range(NL):
                        nc.tensor.matmul(ps2, wk_t[:, l, :], x_t[l],
                                         start=(l == 0), stop=(l == NL - 1))
                    nc.any.tensor_copy(kT[0][:, isl], ps2)
                    ps3 = qkps.tile([2 * H1, 512], f32, tag="qk1ps", name=f"qk1ps{g}")
                    for l in range(NL):
                        nc.tensor.matmul(ps3, wqk1_t[:, l, :], x_t[l],
                                         start=(l == 0), stop=(l == NL - 1))
                    nc.vector.tensor_scalar_add(qT[1][:, isl], ps3[0:H1, :], bq_t[1])
                    nc.any.tensor_copy(kT[1][:, isl], ps3[H1:2 * H1, :])

                def v_proj(g, x_t):
                    c0 = g * 512
                    for mq in range(MQ):
                        wv_t = wvp.tile([128, NL, MQW], f32r, tag="wv",
                                        name=f"wv{g}_{mq}")
                        nc.sync.dma_start(
                            out=wv_t, in_=wv_r[:, :, mq * MQW:(mq + 1) * MQW])
                        for j in range(4 * g, 4 * g + 4):
                            ps = vps.tile([128, MQW], f32, tag="vps",
                                          name=f"vps{g}_{mq}_{j}")
                            jloc = j * 128 - c0
                            for l in range(NL):
                                nc.tensor.matmul(
                                    ps,
                                    x_t[l][:, jloc:jloc + 128],
                                    wv_t[:, l, :],
                                    start=(l == 0), stop=(l == NL - 1))
                            nc.any.tensor_copy(
                                v_t[j][:, mq * MQW:(mq + 1) * MQW], ps)

                # gen 0: q-weights + x first so PE starts ~3us in
                wq_t = wqkp.tile([128, NL, H0], f32r, tag="wq", name="wq")
                nc.sync.dma_start(out=wq_t, in_=wq_d.rearrange("(c p) h -> p c h", p=128))
                x_cur = dma_x_gen(0)
                wk_t = wqkp.tile([128, NL, H0], f32r, tag="wk", name="wk")
                nc.sync.dma_start(out=wk_t, in_=wk_d.rearrange("(c p) h -> p c h", p=128))
                wqk1_t = wqkp.tile([128, NL, 2 * H1], f32r, tag="wqk1", name="wqk1")
                nc.sync.dma_start(out=wqk1_t, in_=wqk1_d.rearrange("(c p) h -> p c h", p=128))
                qk_proj(0, x_cur)
                v_proj(0, x_cur)
                x_next = dma_x_gen(1)
                for g in range(1, 4):
                    x_cur = x_next
                    v_proj(g, x_cur)
                    if g < 3:
                        x_next = dma_x_gen(g + 1)
                    qk_proj(g, x_cur)

            # ---- Phase 3: attention, i-chunk at a time (xT pool is closed)
            with (
                tc.tile_pool(name="ep", bufs=1) as ep,
                tc.tile_pool(name="stg", bufs=3) as stg,
                tc.tile_pool(name="sps", bufs=2, space="PSUM") as sps,
                tc.tile_pool(name="ops", bufs=2, space="PSUM") as ops,
            ):
                for ic in range(NI):
                    isl = slice(ic * 512, (ic + 1) * 512)
                    e_t = []
                    for j in range(NJ):
                        jsl = slice(j * 128, (j + 1) * 128)
                        sp = sps.tile([128, 512], f32, tag="sps")
                        nc.tensor.matmul(sp, kT[0][:, jsl], qT[0][:, isl],
                                         start=True, stop=False)
                        nc.tensor.matmul(sp, kT[1][:, jsl], qT[1][:, isl],
                                         start=False, stop=True)
                        et = ep.tile([128, 512], f32r, tag=f"E{j}")
                        nc.scalar.activation(et, sp,
                                             mybir.ActivationFunctionType.Exp)
                        e_t.append(et)
                    for isub in range(4):
                        i0 = ic * 512 + isub * 128
                        op_t = ops.tile([128, L + 2], f32, tag="ops")
                        esl = slice(isub * 128, (isub + 1) * 128)
                        for j in range(NJ):
                            for (mlo, mhi) in O_CHUNKS:
                                nc.tensor.matmul(
                                    op_t[:, mlo:mhi], e_t[j][:, esl],
                                    v_t[j][:, mlo:mhi],
                                    start=(j == 0), stop=(j == NJ - 1))
                        recip = stg.tile([128, 1], f32, tag="recip")
                        nc.vector.reciprocal(recip, op_t[:, L:L + 1])
                        xr = stg.tile([128, L], f32, tag="xr")
                        nc.sync.dma_start(out=xr, in_=xr_d[i0:i0 + 128, :])
                        ot = stg.tile([128, L], f32, tag="out")
                        nc.vector.scalar_tensor_tensor(
                            out=ot, in0=op_t[:, 0:L], scalar=recip, in1=xr,
                            op0=mybir.AluOpType.mult, op1=mybir.AluOpType.add)
                        nc.sync.dma_start(out=out_d[i0:i0 + 128, :], in_=ot)

    nc.finalize()
    return nc


_NC = None


def _get_nc():
    global _NC
    if _NC is None:
        _NC = _build()
    return _NC


def kernel(x, Wq, bq, Wk, bk, Wv, bv):
    x = np.asarray(x, dtype=np.float32)
    WqT_full = np.asarray(Wq, np.float32).T                    # [L, H]
    WkT_full = np.asarray(Wk, np.float32).T                    # [L, H]
    WqT = np.ascontiguousarray(WqT_full[:, :H0])               # [L, 128]
    WkT = np.ascontiguousarray(WkT_full[:, :H0])               # [L, 128]
    Wqk1T = np.ascontiguousarray(
        np.concatenate([WqT_full[:, H0:], WkT_full[:, H0:]], axis=1))  # [L, 64]
    WvT = np.ascontiguousarray(np.asarray(Wv, np.float32).T)   # [L, L]
    bq = np.asarray(bq, np.float32)
    bv = np.asarray(bv, np.float32)

    nc = _get_nc()
    in_maps = []
    for b in range(B):
        in_maps.append({
            "xT": np.ascontiguousarray(x[b].T),
            "xresid": x[b] + bv[None, :],
            "WqT": WqT,
            "WkT": WkT,
            "Wqk1T": Wqk1T,
            "WvT": WvT,
            "bq": bq,
            "ones": np.ones((128, 2), np.float32),
        })
    res = run_bass_kernel_spmd(nc, in_maps, list(range(B)))
    return np.stack([res.results[b]["out"] for b in range(B)], axis=0)


if __name__ == "__main__":
    rng = np.random.default_rng(0)
    ins = {
        "x": rng.standard_normal((B, N, L)).astype(np.float32),
        "Wq": rng.standard_normal((H, L)).astype(np.float32) * 0.028,
        "bq": rng.standard_normal((H,)).astype(np.float32) * 0.028,
        "Wk": rng.standard_normal((H, L)).astype(np.float32) * 0.028,
        "bk": rng.standard_normal((H,)).astype(np.float32) * 0.028,
        "Wv": rng.standard_normal((L, L)).astype(np.float32) * 0.028,
        "bv": rng.standard_normal((L,)).astype(np.float32) * 0.028,
    }
    out = kernel(**ins)
    print("kernel ran, out shape", out.shape)


# revision 11
# speedup vs baseline: 1.2995x; 1.0390x over previous
"""Bag self-attention kernel for TRN2, data-parallel over the bag dim (8 cores).

Per core (one bag, x: [N=2048, L=1280], H=160):
  q = x@Wq.T + bq ; k = x@Wk.T (bk cancels in softmax) ; v = x@Wv.T
  S = q@k.T ; P = softmax(S) ; out = P@v + (x + bv)      (gamma = 1)

Device-side layout choices:
  - All matmuls run in float32r (TF32-grade, full PE rate at free-dim >= 256).
  - Host pre-transposes x->xT, Wq->WqT, Wk->WkT, Wv->WvT so no on-device
    transposes are needed (sharding is host-side anyway).
  - Attention is computed in the ST=[j,i] layout: ST = kT.T-chunks x qT,
    E = exp(ST) directly feeds P@v as the stationary operand, so the usual
    P-transpose disappears. Softmax runs without max-subtraction (energies
    are ~N(0,160); exp is exact to ~1e-5 over [-90, 80] on ACT) and the
    normalizer Z_i comes for free as a ones-column appended to v.
  - v-bias bv rides the residual (sum_j P == 1), k-bias drops entirely.
"""

import numpy as np

import concourse.bass as bass
import concourse.mybir as mybir
import concourse.tile as tile
from concourse import bacc
from concourse.bass_utils import run_bass_kernel_spmd

B, N, L, H = 8, 2048, 1280, 160
f32 = mybir.dt.float32
f32r = mybir.dt.float32r

NL = L // 128            # 10 l-chunks (contraction for projections)
NJ = N // 128            # 16 j-chunks
NI = N // 512            # 4 i-chunks
MQ = 4                   # WvT window quarters (m = 320 each)
MQW = L // MQ            # 320
H0, H1 = 128, H - 128    # h-chunks 128 + 32
O_CHUNKS = [(0, 512), (512, 1024), (1024, L + 2)]  # last chunk carries 2 Z cols (f32r needs even free dim)


def _build():
    nc = bacc.Bacc()
    xT_d = nc.declare_dram_parameter("xT", [L, N], f32r, isOutput=False)
    xr_d = nc.declare_dram_parameter("xresid", [N, L], f32, isOutput=False)
    wq_d = nc.declare_dram_parameter("WqT", [L, H0], f32r, isOutput=False)
    wk_d = nc.declare_dram_parameter("WkT", [L, H0], f32r, isOutput=False)
    wqk1_d = nc.declare_dram_parameter("Wqk1T", [L, 2 * H1], f32r, isOutput=False)
    wv_d = nc.declare_dram_parameter("WvT", [L, L], f32r, isOutput=False)
    bq_d = nc.declare_dram_parameter("bq", [H], f32, isOutput=False)
    ones_d = nc.declare_dram_parameter("ones", [128, 2], f32r, isOutput=False)
    out_d = nc.declare_dram_parameter("out", [N, L], f32, isOutput=True)

    with tile.TileContext(nc) as tc:
        with (
            tc.tile_pool(name="const", bufs=1) as constp,
            tc.tile_pool(name="vprime", bufs=1) as vpool,
            tc.tile_pool(name="qkt", bufs=1) as qktp,
        ):
            bq_t = [constp.tile([H0, 1], f32, tag="bq0", name="bq0"),
                    constp.tile([H1, 1], f32, tag="bq1", name="bq1")]

            # v' tiles: [128, L+2] per j-chunk; cols L:L+2 are the ones
            # columns for Z (DMA'd lazily inside each generation)
            v_t = [vpool.tile([128, L + 2], f32r, tag=f"v{j}", name=f"v{j}") for j in range(NJ)]

            # qT/kT resident: [160, 2048] as 128+32 partition tiles each
            qT = [qktp.tile([H0, N], f32r, tag="q0", name="q0"),
                  qktp.tile([H1, N], f32r, tag="q1", name="q1")]
            kT = [qktp.tile([H0, N], f32r, tag="k0", name="k0"),
                  qktp.tile([H1, N], f32r, tag="k1", name="k1")]

            # xT consumed in 4 column-generations (gen g: i-chunk g,
            # j-chunks 4g..4g+3); x tiles double-buffered and prefetched one
            # generation ahead. The model serializes DMA transfers, so DMAs
            # are emitted in critical-path order: gen0 weights+x first, then
            # WvT quarters pace the v-projection loop.
            wv_r = wv_d.rearrange("(c p) m -> p c m", p=128)
            with (
                tc.tile_pool(name="wqk", bufs=1) as wqkp,
                tc.tile_pool(name="xt", bufs=2, side="right") as xtp,
                tc.tile_pool(name="wv", bufs=3, side="right") as wvp,
                tc.tile_pool(name="vps", bufs=4, space="PSUM") as vps,
                tc.tile_pool(name="qkps", bufs=1, space="PSUM") as qkps,
            ):
                def dma_x_gen(g):
                    c0 = g * 512
                    ts = []
                    for l in range(NL):
                        t = xtp.tile([128, 512], f32r, tag=f"x{l}", name=f"x{g}_{l}")
                        nc.sync.dma_start(
                            out=t, in_=xT_d[l * 128:(l + 1) * 128, c0:c0 + 512])
                        ts.append(t)
                    return ts

                def qk_proj(g, x_t):
                    isl = slice(g * 512, (g + 1) * 512)
                    ps = qkps.tile([H0, 512], f32, tag="qps", name=f"qps{g}")
                    for l in range(NL):
                        nc.tensor.matmul(ps, wq_t[:, l, :], x_t[l],
                                         start=(l == 0), stop=(l == NL - 1))
                    nc.vector.tensor_scalar_add(qT[0][:, isl], ps, bq_t[0])
                    ps2 = qkps.tile([H0, 512], f32, tag="kps", name=f"kps{g}")
                    for l in range(NL):
                        nc.tensor.matmul(ps2, wk_t[:, l, :], x_t[l],
                                         start=(l == 0), stop=(l == NL - 1))
                    nc.any.tensor_copy(kT[0][:, isl], ps2)
                    ps3 = qkps.tile([2 * H1, 512], f32, tag="qk1ps", name=f"qk1ps{g}")
                    for l in range(NL):
                        nc.tensor.matmul(ps3, wqk1_t[:, l, :], x_t[l],
                                         start=(l == 0), stop=(l == NL - 1))
                    nc.vector.tensor_scalar_add(qT[1][:, isl], ps3[0:H1, :], bq_t[1])
                    nc.any.tensor_copy(kT[1][:, isl], ps3[H1:2 * H1, :])

                def v_proj(g, x_t):
                    c0 = g * 512
                    for mq in range(MQ):
                        wv_t = wvp.tile([128, NL, MQW], f32r, tag="wv",
                                        name=f"wv{g}_{mq}")
                        nc.sync.dma_start(
                            out=wv_t, in_=wv_r[:, :, mq * MQW:(mq + 1) * MQW])
                        for j in range(4 * g, 4 * g + 4):
                            ps = vps.tile([128, MQW], f32, tag="vps",
                                          name=f"vps{g}_{mq}_{j}")
                            jloc = j * 128 - c0
                            for l in range(NL):
                                nc.tensor.matmul(
                                    ps,
                                    x_t[l][:, jloc:jloc + 128],
                                    wv_t[:, l, :],
                                    start=(l == 0), stop=(l == NL - 1))
                            nc.any.tensor_copy(
                                v_t[j][:, mq * MQW:(mq + 1) * MQW], ps)
                    for j in range(4 * g, 4 * g + 4):
                        nc.sync.dma_start(out=v_t[j][:, L:L + 2], in_=ones_d[:])

                # gen 0: q-weights + x first so PE starts ~3us in
                wq_t = wqkp.tile([128, NL, H0], f32r, tag="wq", name="wq")
                nc.sync.dma_start(out=wq_t, in_=wq_d.rearrange("(c p) h -> p c h", p=128))
                x_cur = dma_x_gen(0)
                wk_t = wqkp.tile([128, NL, H0], f32r, tag="wk", name="wk")
                nc.sync.dma_start(out=wk_t, in_=wk_d.rearrange("(c p) h -> p c h", p=128))
                wqk1_t = wqkp.tile([128, NL, 2 * H1], f32r, tag="wqk1", name="wqk1")
                nc.sync.dma_start(out=wqk1_t, in_=wqk1_d.rearrange("(c p) h -> p c h", p=128))
                nc.sync.dma_start(out=bq_t[0], in_=bq_d[0:H0].unsqueeze(1))
                nc.sync.dma_start(out=bq_t[1], in_=bq_d[H0:H].unsqueeze(1))
                qk_proj(0, x_cur)
                v_proj(0, x_cur)
                x_next = dma_x_gen(1)
                for g in range(1, 4):
                    x_cur = x_next
                    v_proj(g, x_cur)
                    if g < 3:
                        x_next = dma_x_gen(g + 1)
                    qk_proj(g, x_cur)

            # ---- Phase 3: attention, i-chunk at a time (xT pool is closed)
            with (
                tc.tile_pool(name="ep", bufs=1) as ep,
                tc.tile_pool(name="stg", bufs=3) as stg,
                tc.tile_pool(name="sps", bufs=2, space="PSUM") as sps,
                tc.tile_pool(name="ops", bufs=2, space="PSUM") as ops,
            ):
                for ic in range(NI):
                    isl = slice(ic * 512, (ic + 1) * 512)
                    e_t = []
                    for j in range(NJ):
                        jsl = slice(j * 128, (j + 1) * 128)
                        sp = sps.tile([128, 512], f32, tag="sps")
                        nc.tensor.matmul(sp, kT[0][:, jsl], qT[0][:, isl],
                                         start=True, stop=False)
                        nc.tensor.matmul(sp, kT[1][:, jsl], qT[1][:, isl],
                                         start=False, stop=True)
                        et = ep.tile([128, 512], f32r, tag=f"E{j}")
                        nc.scalar.activation(et, sp,
                                             mybir.ActivationFunctionType.Exp)
                        e_t.append(et)
                    for isub in range(4):
                        i0 = ic * 512 + isub * 128
                        op_t = ops.tile([128, L + 2], f32, tag="ops")
                        esl = slice(isub * 128, (isub + 1) * 128)
                        for j in range(NJ):
                            for (mlo, mhi) in O_CHUNKS:
                                nc.tensor.matmul(
                                    op_t[:, mlo:mhi], e_t[j][:, esl],
                                    v_t[j][:, mlo:mhi],
                                    start=(j == 0), stop=(j == NJ - 1))
                        recip = stg.tile([128, 1], f32, tag="recip")
                        nc.vector.reciprocal(recip, op_t[:, L:L + 1])
                        xr = stg.tile([128, L], f32, tag="xr")
                        nc.sync.dma_start(out=xr, in_=xr_d[i0:i0 + 128, :])
                        ot = stg.tile([128, L], f32, tag="out")
                        nc.vector.scalar_tensor_tensor(
                            out=ot, in0=op_t[:, 0:L], scalar=recip, in1=xr,
                            op0=mybir.AluOpType.mult, op1=mybir.AluOpType.add)
                        nc.sync.dma_start(out=out_d[i0:i0 + 128, :], in_=ot)

    nc.finalize()
    return nc


_NC = None


def _get_nc():
    global _NC
    if _NC is None:
        _NC = _build()
    return _NC


def kernel(x, Wq, bq, Wk, bk, Wv, bv):
    x = np.asarray(x, dtype=np.float32)
    WqT_full = np.asarray(Wq, np.float32).T                    # [L, H]
    WkT_full = np.asarray(Wk, np.float32).T                    # [L, H]
    WqT = np.ascontiguousarray(WqT_full[:, :H0])               # [L, 128]
    WkT = np.ascontiguousarray(WkT_full[:, :H0])               # [L, 128]
    Wqk1T = np.ascontiguousarray(
        np.concatenate([WqT_full[:, H0:], WkT_full[:, H0:]], axis=1))  # [L, 64]
    WvT = np.ascontiguousarray(np.asarray(Wv, np.float32).T)   # [L, L]
    bq = np.asarray(bq, np.float32)
    bv = np.asarray(bv, np.float32)

    nc = _get_nc()
    in_maps = []
    for b in range(B):
        in_maps.append({
            "xT": np.ascontiguousarray(x[b].T),
            "xresid": x[b] + bv[None, :],
            "WqT": WqT,
            "WkT": WkT,
            "Wqk1T": Wqk1T,
            "WvT": WvT,
            "bq": bq,
            "ones": np.ones((128, 2), np.float32),
        })
    res = run_bass_kernel_spmd(nc, in_maps, list(range(B)))
    return np.stack([res.results[b]["out"] for b in range(B)], axis=0)


if __name__ == "__main__":
    rng = np.random.default_rng(0)
    ins = {
        "x": rng.standard_normal((B, N, L)).astype(np.float32),
        "Wq": rng.standard_normal((H, L)).astype(np.float32) * 0.028,
        "bq": rng.standard_normal((H,)).astype(np.float32) * 0.028,
        "Wk": rng.standard_normal((H, L)).astype(np.float32) * 0.028,
        "bk": rng.standard_normal((H,)).astype(np.float32) * 0.028,
        "Wv": rng.standard_normal((L, L)).astype(np.float32) * 0.028,
        "bv": rng.standard_normal((L,)).astype(np.float32) * 0.028,
    }
    out = kernel(**ins)
    print("kernel ran, out shape", out.shape)
